# revision 25
# baseline (speedup 1.0000x reference)
"""Trainium2 Bass kernel for masked-softmax attention (sparse_attention).

reference:
    S = Q @ K^T / sqrt(128)            # [N, nq, nk]
    A = softmax(S, axis=-1) * mask
    A = A / (sum_k A + 1e-6)
    O = A @ V

Device identity (softmax normalizer cancels in the renormalization):
    E = exp(S); P = E * mask
    O[q, :] = (P @ V)[q, :] / sum_k P[q, k]
(the reference's +1e-6 is ~2e-6 relative to the masked sum and dropped).

Sharding: N=32 batch-heads split across 8 NeuronCores, 4 per core; no
cross-core communication. Host does layout-only staging: Q/K transposed to
[d, n] for the TensorEngine contraction-on-partitions layout, V tiled,
mask transposed to [k, q] and cast to bf16 (the kernel multiplies it in
bf16 either way), output returned bf16 and upcast on host.

Per-core pipeline, per (batch b, q-half h of 1024):
  k-phase, per k-tile kt (128 rows of K):
    mm1  (PE, f32r): ST_kt = KT_kt.T @ QT[:, h]   [128k x 1024q] -> PSUM
    exp  (ACT):      ET = exp(ST/sqrt(d)) [bf16]                 -> SBUF
    mult (DVE, 2x):  PT[kt] = ET * maskT_kt [bf16]               -> P^T slab
  q-phase, per q-tile qc (8 per half):
    mm2  (PE, bf16): O|denom = sum_kt PT[kt][:,qc].T @ [V_kt | 1] -> PSUM
    recip+scale (DVE): st[qc] = O * (1/denom)  [bf16]
  store st -> out (natural [q, d] tiling) on the ACT HWDGE ring.
"""
import sys

sys.path.insert(0, "/opt/trn_rl_repo")

import ml_dtypes
import numpy as np

from concourse import bacc, mybir, tile
from concourse.bass_utils import run_bass_kernel_spmd

N, NQ, NK, D = 32, 2048, 2048, 128
N_CORES = 8
B = N // N_CORES          # batches per core
QT_TILES = NQ // 128      # q tiles per batch
KT_TILES = NK // 128      # k tiles per batch
QH = NQ // 2              # q-half width
SCALE = float(1.0 / np.sqrt(D))

F32 = mybir.dt.float32
F32R = mybir.dt.float32r
BF16 = mybir.dt.bfloat16

_cached = {}


def build():
    if "nc" in _cached:
        return _cached["nc"]
    nc = bacc.Bacc("TRN2", target_bir_lowering=False, debug=False)

    qt_d = nc.dram_tensor("queriesT", [B, D, NQ], F32, kind="ExternalInput").ap()
    kt_d = nc.dram_tensor("keysT", [B, D, NK], F32, kind="ExternalInput").ap()
    v_d = nc.dram_tensor("valuesP", [B, 128, KT_TILES, D], BF16, kind="ExternalInput").ap()
    m_d = nc.dram_tensor("maskT", [B, KT_TILES, 128, NQ], BF16, kind="ExternalInput").ap()
    o_d = nc.dram_tensor("out", [B, 128, QT_TILES, D], BF16, kind="ExternalOutput").ap()

    with tile.TileContext(nc) as tc:
        with (
            tc.tile_pool(name="nat", bufs=2) as natpool,
            tc.tile_pool(name="tr", bufs=2) as trpool,
            tc.tile_pool(name="vbo", bufs=2) as vpool,
            tc.tile_pool(name="maskc", bufs=4) as mpool,
            tc.tile_pool(name="work", bufs=3) as wpool,
            tc.tile_pool(name="ptslab", bufs=2) as ptpool,
            tc.tile_pool(name="stage", bufs=2) as stpool,
            tc.tile_pool(name="spsum", bufs=3, space="PSUM") as spool,
            tc.tile_pool(name="opsum", bufs=2, space="PSUM") as opool,
        ):
            def q_iter(prev, qc, part, o_prev):
                """Half of one q-tile's accumulation chain (k-tiles 8*part..).

                Returns the o_ps tile so part 1 can continue the chain.
                """
                pt, vb, st, b, h = prev
                qtile = h * (QT_TILES // 2) + qc
                o_ps = o_prev if part else opool.tile([128, D + 1], F32, tag="o")
                half = KT_TILES // 2
                for kt in range(part * half, (part + 1) * half):
                    nc.tensor.matmul(
                        o_ps[:],
                        pt[:, kt, qc * 128:(qc + 1) * 128],
                        vb[:, kt, :],
                        start=(kt == 0),
                        stop=(kt == KT_TILES - 1),
                    )
                if part == 1:
                    rd = wpool.tile([128, 1], F32, tag="rd")
                    nc.vector.reciprocal(rd[:], o_ps[:, D:D + 1])
                    nc.vector.tensor_scalar_mul(
                        st[:, qtile, :], o_ps[:, 0:D], rd[:]
                    )
                    if qtile == QT_TILES - 1:
                        nc.scalar.dma_start(o_d[b], st[:])
                return o_ps

            prev = None
            o_carry = None
            for b in range(B):
                kt_sb = trpool.tile([128, NK], F32R, tag="kt")
                qt_sb = trpool.tile([128, NQ], F32R, tag="qt")
                if b == 0:
                    # split the first loads so the first mm1 starts sooner
                    for c4 in range(4):
                        sl4 = slice(c4 * (NK // 4), (c4 + 1) * (NK // 4))
                        nc.sync.dma_start(
                            kt_sb[:, sl4], kt_d[b, :, sl4].bitcast(F32R)
                        )
                        nc.sync.dma_start(
                            qt_sb[:, sl4], qt_d[b, :, sl4].bitcast(F32R)
                        )
                else:
                    nc.sync.dma_start(kt_sb[:], kt_d[b].bitcast(F32R))
                    nc.sync.dma_start(qt_sb[:], qt_d[b].bitcast(F32R))
                vnb = natpool.tile([128, KT_TILES, D], BF16, tag="vn")
                nc.sync.dma_start(vnb[:], v_d[b])
                vb = vpool.tile([128, KT_TILES, D + 1], BF16, tag="vb")
                nc.vector.tensor_copy(vb[:, :, 0:D], vnb[:])
                nc.vector.memset(vb[:, :, D], 1.0)

                st = stpool.tile([128, QT_TILES, D], BF16, tag="st")

                for h in range(2):
                    pt = ptpool.tile([128, KT_TILES, QH], BF16, tag="pt")
                    for kt in range(KT_TILES):
                        mask_c = mpool.tile([128, QH], BF16, tag="mc")
                        nc.sync.dma_start(
                            mask_c[:], m_d[b, kt, :, h * QH:(h + 1) * QH]
                        )

                        s_ps = spool.tile([128, QH], F32, tag="s")
                        for c in range(2):
                            nc.tensor.matmul(
                                s_ps[:, c * 512:(c + 1) * 512],
                                kt_sb[:, kt * 128:(kt + 1) * 128],
                                qt_sb[:, h * QH + c * 512: h * QH + (c + 1) * 512],
                                start=True,
                                stop=True,
                            )
                        e_sb = wpool.tile([128, QH], BF16, tag="e")
                        nc.scalar.activation(
                            e_sb[:],
                            s_ps[:],
                            mybir.ActivationFunctionType.Exp,
                            scale=SCALE,
                        )
                        nc.vector.tensor_tensor(
                            out=pt[:, kt, :],
                            in0=e_sb[:],
                            in1=mask_c[:],
                            op=mybir.AluOpType.mult,
                        )
                        # interleave the previous slab's q-phase into this
                        # k-phase so the PE never waits on a full slab
                        if prev is not None:
                            o_carry = q_iter(prev, kt // 2, kt % 2, o_carry)
                    prev = (pt, vb, st, b, h)

            for qc in range(QT_TILES // 2):
                o_carry = q_iter(prev, qc, 0, o_carry)
                o_carry = q_iter(prev, qc, 1, o_carry)

    nc.compile()
    _cached["nc"] = nc
    return nc


def kernel(queries, keys, values, mask, _trace=False, **kw):
    nc = build()
    in_maps = []
    for c in range(N_CORES):
        sl = slice(c * B, (c + 1) * B)
        in_maps.append(
            {
                "queriesT": np.ascontiguousarray(queries[sl].transpose(0, 2, 1)),
                "keysT": np.ascontiguousarray(keys[sl].transpose(0, 2, 1)),
                "valuesP": np.ascontiguousarray(
                    values[sl].reshape(B, KT_TILES, 128, D).transpose(0, 2, 1, 3)
                ).astype(ml_dtypes.bfloat16),
                "maskT": np.ascontiguousarray(
                    mask[sl].transpose(0, 2, 1)
                ).astype(ml_dtypes.bfloat16).reshape(B, KT_TILES, 128, NQ),
            }
        )
    res = run_bass_kernel_spmd(
        nc, in_maps, core_ids=list(range(N_CORES)), trace=_trace
    )
    out = np.concatenate(
        [
            res.results[c]["out"]
            .astype(np.float32)
            .transpose(0, 2, 1, 3)
            .reshape(B, NQ, D)
            for c in range(N_CORES)
        ],
        axis=0,
    )
    if _trace:
        return out, res
    return out


# revision 26
# speedup vs baseline: 1.0372x; 1.0372x over previous
"""Trainium2 Bass kernel for masked-softmax attention (sparse_attention).

reference:
    S = Q @ K^T / sqrt(128)            # [N, nq, nk]
    A = softmax(S, axis=-1) * mask
    A = A / (sum_k A + 1e-6)
    O = A @ V

Device identity (softmax normalizer cancels in the renormalization):
    E = exp(S); P = E * mask
    O[q, :] = (P @ V)[q, :] / sum_k P[q, k]
(the reference's +1e-6 is ~2e-6 relative to the masked sum and dropped).

Sharding: N=32 batch-heads split across 8 NeuronCores, 4 per core; no
cross-core communication. Host does layout-only staging: Q/K transposed to
[d, n] for the TensorEngine contraction-on-partitions layout, V tiled,
mask transposed to [k, q] and cast to bf16 (the kernel multiplies it in
bf16 either way), output returned bf16 and upcast on host.

Per-core pipeline, per (batch b, q-half h of 1024):
  k-phase, per k-tile kt (128 rows of K):
    mm1  (PE, f32r): ST_kt = KT_kt.T @ QT[:, h]   [128k x 1024q] -> PSUM
    exp  (ACT):      ET = exp(ST/sqrt(d)) [bf16]                 -> SBUF
    mult (DVE, 2x):  PT[kt] = ET * maskT_kt [bf16]               -> P^T slab
  q-phase, per q-tile qc (8 per half):
    mm2  (PE, bf16): O|denom = sum_kt PT[kt][:,qc].T @ [V_kt | 1] -> PSUM
    recip+scale (DVE): st[qc] = O * (1/denom)  [bf16]
  store st -> out (natural [q, d] tiling) on the ACT HWDGE ring.
"""
import sys

sys.path.insert(0, "/opt/trn_rl_repo")

import ml_dtypes
import numpy as np

from concourse import bacc, mybir, tile
from concourse.bass_utils import run_bass_kernel_spmd

N, NQ, NK, D = 32, 2048, 2048, 128
N_CORES = 8
B = N // N_CORES          # batches per core
QT_TILES = NQ // 128      # q tiles per batch
KT_TILES = NK // 128      # k tiles per batch
QH = NQ // 2              # q-half width
SCALE = float(1.0 / np.sqrt(D))

F32 = mybir.dt.float32
F32R = mybir.dt.float32r
BF16 = mybir.dt.bfloat16

_cached = {}


def build():
    if "nc" in _cached:
        return _cached["nc"]
    nc = bacc.Bacc("TRN2", target_bir_lowering=False, debug=False)

    qt_d = nc.dram_tensor("queriesT", [B, D, NQ], F32, kind="ExternalInput").ap()
    kt_d = nc.dram_tensor("keysT", [B, D, NK], F32, kind="ExternalInput").ap()
    v_d = nc.dram_tensor("valuesP", [B, 128, KT_TILES, D], BF16, kind="ExternalInput").ap()
    m_d = nc.dram_tensor("maskT", [B, KT_TILES, 128, NQ], BF16, kind="ExternalInput").ap()
    o_d = nc.dram_tensor("out", [B, 128, QT_TILES, D], BF16, kind="ExternalOutput").ap()

    with tile.TileContext(nc) as tc:
        with (
            tc.tile_pool(name="nat", bufs=2) as natpool,
            tc.tile_pool(name="tr", bufs=2) as trpool,
            tc.tile_pool(name="vbo", bufs=2) as vpool,
            tc.tile_pool(name="maskc", bufs=4) as mpool,
            tc.tile_pool(name="work", bufs=3) as wpool,
            tc.tile_pool(name="ptslab", bufs=2) as ptpool,
            tc.tile_pool(name="stage", bufs=2) as stpool,
            tc.tile_pool(name="spsum", bufs=3, space="PSUM") as spool,
            tc.tile_pool(name="opsum", bufs=2, space="PSUM") as opool,
        ):
            def q_iter(prev, qc):
                """One q-tile of the q-phase for a finished P^T slab."""
                pt, vb, st, b, h = prev
                qtile = h * (QT_TILES // 2) + qc
                o_ps = opool.tile([128, D + 1], F32, tag="o")
                for kt in range(KT_TILES):
                    nc.tensor.matmul(
                        o_ps[:],
                        pt[:, kt, qc * 128:(qc + 1) * 128],
                        vb[:, kt, :],
                        start=(kt == 0),
                        stop=(kt == KT_TILES - 1),
                    )
                rd = wpool.tile([128, 1], F32, tag="rd")
                nc.vector.reciprocal(rd[:], o_ps[:, D:D + 1])
                nc.vector.tensor_scalar_mul(st[:, qtile, :], o_ps[:, 0:D], rd[:])
                if qtile == QT_TILES - 1:
                    nc.scalar.dma_start(o_d[b], st[:])

            prev = None
            for b in range(B):
                kt_sb = trpool.tile([128, NK], F32R, tag="kt")
                qt_sb = trpool.tile([128, NQ], F32R, tag="qt")
                if b == 0:
                    # split the first loads so the first mm1 starts sooner
                    for c4 in range(4):
                        sl4 = slice(c4 * (NK // 4), (c4 + 1) * (NK // 4))
                        nc.sync.dma_start(
                            kt_sb[:, sl4], kt_d[b, :, sl4].bitcast(F32R)
                        )
                        nc.sync.dma_start(
                            qt_sb[:, sl4], qt_d[b, :, sl4].bitcast(F32R)
                        )
                else:
                    nc.sync.dma_start(kt_sb[:], kt_d[b].bitcast(F32R))
                    nc.sync.dma_start(qt_sb[:], qt_d[b].bitcast(F32R))
                vnb = natpool.tile([128, KT_TILES, D], BF16, tag="vn")
                nc.sync.dma_start(vnb[:], v_d[b])
                vb = vpool.tile([128, KT_TILES, D + 1], BF16, tag="vb")
                nc.vector.tensor_copy(vb[:, :, 0:D], vnb[:])
                nc.vector.memset(vb[:, :, D], 1.0)

                st = stpool.tile([128, QT_TILES, D], BF16, tag="st")

                for h in range(2):
                    pt = ptpool.tile([128, KT_TILES, QH], BF16, tag="pt")
                    for kt in range(KT_TILES):
                        mask_c = mpool.tile([128, QH], BF16, tag="mc")
                        nc.sync.dma_start(
                            mask_c[:], m_d[b, kt, :, h * QH:(h + 1) * QH]
                        )

                        s_ps = spool.tile([128, QH], F32, tag="s")
                        for c in range(2):
                            nc.tensor.matmul(
                                s_ps[:, c * 512:(c + 1) * 512],
                                kt_sb[:, kt * 128:(kt + 1) * 128],
                                qt_sb[:, h * QH + c * 512: h * QH + (c + 1) * 512],
                                start=True,
                                stop=True,
                            )
                        e_sb = wpool.tile([128, QH], BF16, tag="e")
                        nc.scalar.activation(
                            e_sb[:],
                            s_ps[:],
                            mybir.ActivationFunctionType.Exp,
                            scale=SCALE,
                        )
                        nc.vector.tensor_tensor(
                            out=pt[:, kt, :],
                            in0=e_sb[:],
                            in1=mask_c[:],
                            op=mybir.AluOpType.mult,
                        )
                        # interleave the previous slab's q-phase into this
                        # k-phase so the PE never waits on a full slab
                        if prev is not None and kt % 2 == 1:
                            q_iter(prev, kt // 2)
                    prev = (pt, vb, st, b, h)

            for qc in range(QT_TILES // 2):
                q_iter(prev, qc)

    nc.compile()
    _cached["nc"] = nc
    return nc


def kernel(queries, keys, values, mask, _trace=False, **kw):
    nc = build()
    in_maps = []
    for c in range(N_CORES):
        sl = slice(c * B, (c + 1) * B)
        in_maps.append(
            {
                "queriesT": np.ascontiguousarray(queries[sl].transpose(0, 2, 1)),
                "keysT": np.ascontiguousarray(keys[sl].transpose(0, 2, 1)),
                "valuesP": np.ascontiguousarray(
                    values[sl].reshape(B, KT_TILES, 128, D).transpose(0, 2, 1, 3)
                ).astype(ml_dtypes.bfloat16),
                "maskT": np.ascontiguousarray(
                    mask[sl].transpose(0, 2, 1)
                ).astype(ml_dtypes.bfloat16).reshape(B, KT_TILES, 128, NQ),
            }
        )
    res = run_bass_kernel_spmd(
        nc, in_maps, core_ids=list(range(N_CORES)), trace=_trace
    )
    out = np.concatenate(
        [
            res.results[c]["out"]
            .astype(np.float32)
            .transpose(0, 2, 1, 3)
            .reshape(B, NQ, D)
            for c in range(N_CORES)
        ],
        axis=0,
    )
    if _trace:
        return out, res
    return out
